# revision 6
# baseline (speedup 1.0000x reference)
"""CRF decoder (logZ - gold) Trainium2 kernel.

Strategy (hardcoded for B=64, S=1024, C=1, N=256, 8 cores):
- Data-parallel over batch: 8 sequences per core.
- Log-semiring forward scan done in *linear* space with a constant host-side
  log-scale sigma = log(256)+0.5 subtracted from each emission, so the scaled
  probabilities p_t stay within fp32/bf16 exponent range for all 1024 steps
  (drift is a mean-zero random walk, ~±3 nats) — no device renormalization.
- Per step: u = W^T p (4 bf16 128x128 matmuls, PSUM fp32), p' = u * E_t (DVE),
  where W = exp(transitions), E_t = exp(em_t - sigma) (ScalarE bulk exp).
- Variable lengths: per-step scalar z_t[b] = p_t . exp(last) via a 1-column
  matmul; host reads z at t = len_b - 1 and assembles
  logZ_b = log z_{len-1} + (len-1)*sigma.  No per-step masking on device.
- Gold emission score on device: one-hot (host-built, masked) times raw
  emissions, multiply+reduce per chunk on DVE, partition-sum on host (tiny).
- Gold transition/head/last scores touch only the tiny parameter tensors and
  targets; computed on host.
"""

import math
from contextlib import ExitStack

import numpy as np
import ml_dtypes

import concourse.bass as bass
import concourse.tile as tile
from concourse import bacc, mybir
from concourse.bass_utils import run_bass_kernel_spmd

B, S, N = 64, 1024, 256
NCORES = 8
BL = B // NCORES  # 8 sequences per core
TC = 128          # time-chunk length
NCHUNK = S // TC
SIGMA = math.log(256.0) + 0.5
ZMIN = 383        # earliest t for which z_t is recorded (lengths >= ZMIN+2 expected)

F32 = mybir.dt.float32
BF16 = mybir.dt.bfloat16


def _crf_tile_kernel(ctx: ExitStack, tc: tile.TileContext, aps: dict):
    nc = tc.nc
    em_d, oh_d = aps["em"], aps["oh"]          # [2,128,S,BL] bf16 dram
    w_d = aps["w"]                              # [2,128,2,128] bf16
    el_d = aps["el"]                            # [2,128,1] bf16
    hd_d = aps["hd"]                            # [2,128,1] f32
    zh_d = aps["zhist"]                         # [1, S*BL] f32 out
    ea_d = aps["emitacc"]                       # [128, 2*BL] f32 out

    consts = ctx.enter_context(tc.tile_pool(name="consts", bufs=1))
    state = ctx.enter_context(tc.tile_pool(name="state", bufs=1))
    empool = ctx.enter_context(tc.tile_pool(name="em", bufs=4))
    ohpool = ctx.enter_context(tc.tile_pool(name="oh", bufs=4))
    epool = ctx.enter_context(tc.tile_pool(name="E", bufs=4))
    tmppool = ctx.enter_context(tc.tile_pool(name="tmp", bufs=2))
    redpool = ctx.enter_context(tc.tile_pool(name="red", bufs=2))
    upool = ctx.enter_context(tc.tile_pool(name="u", bufs=4, space="PSUM"))
    zpool = ctx.enter_context(tc.tile_pool(name="z", bufs=2, space="PSUM"))

    # ---- constants into SBUF ----
    w_sb = []   # w_sb[ih][:, jh, :] = W[ih*128:(ih+1)*128, jh*128:(jh+1)*128]
    for ih in range(2):
        t_ = consts.tile([128, 2, 128], BF16, name=f"w{ih}", tag=f"w{ih}")
        nc.sync.dma_start(out=t_[:], in_=w_d[ih])
        w_sb.append(t_)
    el_sb = []
    hd_sb = []
    for ih in range(2):
        e_ = consts.tile([128, 1], BF16, name=f"el{ih}", tag=f"el{ih}")
        nc.sync.dma_start(out=e_[:], in_=el_d[ih])
        el_sb.append(e_)
        h_ = consts.tile([128, 1], F32, name=f"hd{ih}", tag=f"hd{ih}")
        nc.sync.dma_start(out=h_[:], in_=hd_d[ih])
        hd_sb.append(h_)

    sig_sb = consts.tile([128, 1], F32, name="sigb", tag="sigb")
    nc.vector.memset(sig_sb[:], -SIGMA)

    # persistent state: ping-pong p tiles per half
    p_sb = [[state.tile([128, BL], BF16, name=f"p{par}{ih}", tag=f"p{par}{ih}") for ih in range(2)]
            for par in range(2)]
    # z history on one partition + emission-score accumulators
    zhist = consts.tile([1, S * BL], F32, name="zhist", tag="zhist")
    nc.vector.memset(zhist[:], 0.0)
    acc = [consts.tile([128, BL], F32, name=f"acc{ih}", tag=f"acc{ih}") for ih in range(2)]
    for ih in range(2):
        nc.vector.memset(acc[ih][:], 0.0)

    alu = mybir.AluOpType

    def do_z(tt, p_pair):
        """z_tt = expLast . p_tt -> zhist[tt]"""
        z = zpool.tile([1, BL], F32, name="z", tag="z")
        nc.tensor.matmul(z[:], el_sb[0][:], p_pair[0][:], start=True, stop=False)
        nc.tensor.matmul(z[:], el_sb[1][:], p_pair[1][:], start=False, stop=True)
        nc.vector.tensor_copy(zhist[:, tt * BL:(tt + 1) * BL], z[:])

    for c in range(NCHUNK):
        em_sb, oh_sb, e_sb = [], [], []
        for ih in range(2):
            em_t = empool.tile([128, TC, BL], BF16, name=f"emt{ih}", tag=f"em{ih}")
            nc.sync.dma_start(out=em_t[:], in_=em_d[ih, :, c * TC:(c + 1) * TC, :])
            em_sb.append(em_t)
            oh_t = ohpool.tile([128, TC, BL], BF16, name=f"oht{ih}", tag=f"oh{ih}")
            nc.sync.dma_start(out=oh_t[:], in_=oh_d[ih, :, c * TC:(c + 1) * TC, :])
            oh_sb.append(oh_t)
            e_t = epool.tile([128, TC, BL], BF16, name=f"Et{ih}", tag=f"E{ih}")
            nc.scalar.activation(e_t[:], em_t[:],
                                 mybir.ActivationFunctionType.Exp,
                                 bias=sig_sb[:], scale=1.0)
            e_sb.append(e_t)

        if c == 0:
            # p_0 = exp(head + em_0)
            for ih in range(2):
                nc.scalar.activation(p_sb[0][ih][:], em_sb[ih][:, 0, :],
                                     mybir.ActivationFunctionType.Exp,
                                     bias=hd_sb[ih][:], scale=1.0)

        # ---- emission gold score for this chunk (DVE, off critical path) ----
        for ih in range(2):
            tmp = tmppool.tile([128, TC, BL], BF16, name=f"tmpt{ih}", tag=f"tmp{ih}")
            nc.vector.tensor_mul(tmp[:], em_sb[ih][:], oh_sb[ih][:])
            red = redpool.tile([128, BL], F32, name=f"redt{ih}", tag=f"red{ih}")
            # reduce over t (innermost after rearrange)
            nc.vector.tensor_reduce(red[:], tmp[:].rearrange("p t b -> p b t"),
                                    mybir.AxisListType.X, alu.add)
            nc.vector.tensor_add(acc[ih][:], acc[ih][:], red[:])

        # ---- the scan steps of this chunk ----
        for r in range(TC):
            t = c * TC + r
            if t == 0:
                continue
            pa = p_sb[(t - 1) % 2]
            pb = p_sb[t % 2]
            u_lo = upool.tile([128, BL], F32, name="u", tag="u")
            u_hi = upool.tile([128, BL], F32, name="u", tag="u")
            nc.tensor.matmul(u_lo[:], w_sb[0][:, 0, :], pa[0][:], start=True, stop=False)
            nc.tensor.matmul(u_lo[:], w_sb[1][:, 0, :], pa[1][:], start=False, stop=True)
            nc.tensor.matmul(u_hi[:], w_sb[0][:, 1, :], pa[0][:], start=True, stop=False)
            nc.tensor.matmul(u_hi[:], w_sb[1][:, 1, :], pa[1][:], start=False, stop=True)
            if t - 1 >= ZMIN:
                do_z(t - 1, pa)
            nc.vector.tensor_mul(pb[0][:], u_lo[:], e_sb[0][:, r, :])
            nc.vector.tensor_mul(pb[1][:], u_hi[:], e_sb[1][:, r, :])

    do_z(S - 1, p_sb[(S - 1) % 2])

    # ---- outputs ----
    nc.sync.dma_start(out=zh_d[:], in_=zhist[:])
    nc.sync.dma_start(out=ea_d[:, 0:BL], in_=acc[0][:])
    nc.sync.dma_start(out=ea_d[:, BL:2 * BL], in_=acc[1][:])


_NC_CACHE = None


def _build_nc():
    global _NC_CACHE
    if _NC_CACHE is not None:
        return _NC_CACHE
    nc = bacc.Bacc("TRN2", target_bir_lowering=False, debug=False,
                   num_devices=NCORES)
    aps = {
        "em": nc.dram_tensor("em", [2, 128, S, BL], BF16, kind="ExternalInput").ap(),
        "oh": nc.dram_tensor("oh", [2, 128, S, BL], BF16, kind="ExternalInput").ap(),
        "w": nc.dram_tensor("w", [2, 128, 2, 128], BF16, kind="ExternalInput").ap(),
        "el": nc.dram_tensor("el", [2, 128, 1], BF16, kind="ExternalInput").ap(),
        "hd": nc.dram_tensor("hd", [2, 128, 1], F32, kind="ExternalInput").ap(),
        "zhist": nc.dram_tensor("zhist", [1, S * BL], F32, kind="ExternalOutput").ap(),
        "emitacc": nc.dram_tensor("emitacc", [128, 2 * BL], F32, kind="ExternalOutput").ap(),
    }
    with tile.TileContext(nc) as tc:
        with ExitStack() as ctx:
            _crf_tile_kernel(ctx, tc, aps)
    nc.compile()
    _NC_CACHE = nc
    return nc


def _host_gold_small(targets, lengths, transitions, head_transitions, last_transitions):
    """Transition/head/last parts of the gold score (no big-tensor access)."""
    T = transitions[0].astype(np.float64)
    tr = T[targets[:, :-1], targets[:, 1:]]                       # [B,S-1]
    pmask = (np.arange(1, S)[None, :] < lengths[:, None])
    trans_score = (tr * pmask).sum(1)
    head_score = head_transitions[0][targets[:, 0]].astype(np.float64)
    last_tag = np.take_along_axis(targets, (lengths - 1)[:, None], axis=1)[:, 0]
    last_score = last_transitions[0][last_tag].astype(np.float64)
    return trans_score + head_score + last_score


def _make_in_maps(inputs):
    emissions = np.asarray(inputs["emissions"])
    targets = np.asarray(inputs["targets"])
    lengths = np.asarray(inputs["lengths"])
    transitions = np.asarray(inputs["transitions"])
    head_transitions = np.asarray(inputs["head_transitions"])
    last_transitions = np.asarray(inputs["last_transitions"])

    W = np.exp(transitions[0].astype(np.float64)).astype(ml_dtypes.bfloat16)
    w_sh = np.ascontiguousarray(W.reshape(2, 128, 2, 128))
    el_sh = np.ascontiguousarray(
        np.exp(last_transitions[0].astype(np.float64))
        .astype(ml_dtypes.bfloat16).reshape(2, 128, 1))
    hd_sh = np.ascontiguousarray(
        head_transitions[0].astype(np.float32).reshape(2, 128, 1))

    em_bf = emissions[:, :, 0, :].astype(ml_dtypes.bfloat16)      # [B,S,N]

    in_maps = []
    for c in range(NCORES):
        sl = slice(c * BL, (c + 1) * BL)
        em_c = np.ascontiguousarray(
            em_bf[sl].transpose(2, 1, 0).reshape(2, 128, S, BL))  # [j,t,b]
        tgt_c = targets[sl]                                       # [BL,S]
        len_c = lengths[sl]
        oh_c = np.zeros((N, S, BL), dtype=ml_dtypes.bfloat16)
        bb, tt = np.meshgrid(np.arange(BL), np.arange(S), indexing="ij")
        valid = tt < len_c[:, None]
        oh_c[tgt_c[bb[valid], tt[valid]], tt[valid], bb[valid]] = 1.0
        oh_c = np.ascontiguousarray(oh_c.reshape(2, 128, S, BL))
        in_maps.append({"em": em_c, "oh": oh_c, "w": w_sh, "el": el_sh,
                        "hd": hd_sh})
    return in_maps


def kernel(emissions, targets, lengths, transitions, head_transitions,
           last_transitions):
    emissions = np.asarray(emissions)
    targets = np.asarray(targets)
    lengths = np.asarray(lengths)
    transitions = np.asarray(transitions)
    head_transitions = np.asarray(head_transitions)
    last_transitions = np.asarray(last_transitions)
    assert emissions.shape == (B, S, 1, N), emissions.shape

    nc = _build_nc()
    in_maps = _make_in_maps(dict(
        emissions=emissions, targets=targets, lengths=lengths,
        transitions=transitions, head_transitions=head_transitions,
        last_transitions=last_transitions))

    res = run_bass_kernel_spmd(nc, in_maps, list(range(NCORES)))

    logZ = np.zeros(B, np.float64)
    emit = np.zeros(B, np.float64)
    tstar = lengths - 1
    for c in range(NCORES):
        zh = res.results[c]["zhist"].reshape(S, BL).astype(np.float64)
        ea = res.results[c]["emitacc"].astype(np.float64)         # [128, 2*BL]
        for bl in range(BL):
            b = c * BL + bl
            logZ[b] = np.log(zh[tstar[b], bl]) + tstar[b] * SIGMA
            emit[b] = ea[:, bl].sum() + ea[:, BL + bl].sum()

    gold = emit + _host_gold_small(targets, lengths, transitions,
                                   head_transitions, last_transitions)
    out = (logZ - gold).astype(np.float32)[:, None]               # [B, C=1]

    # safety: exact host fallback for any sequence whose z was not recorded
    bad = np.nonzero(tstar < ZMIN)[0]
    if bad.size:
        out[bad, 0] = _host_exact(emissions, targets, lengths, transitions,
                                  head_transitions, last_transitions, bad)
    return out


def _host_exact(emissions, targets, lengths, transitions, head_transitions,
                last_transitions, idx):
    from scipy.special import logsumexp  # noqa — tiny fallback, rarely used
    outs = []
    T = transitions[0].astype(np.float64)
    for b in idx:
        L = int(lengths[b])
        em = emissions[b, :, 0, :].astype(np.float64)
        alpha = head_transitions[0].astype(np.float64) + em[0]
        for t in range(1, L):
            alpha = logsumexp(alpha[:, None] + T, axis=0) + em[t]
        logZ = logsumexp(alpha + last_transitions[0].astype(np.float64))
        tgt = targets[b]
        gold = em[np.arange(L), tgt[:L]].sum()
        gold += T[tgt[:L - 1], tgt[1:L]].sum()
        gold += head_transitions[0][tgt[0]] + last_transitions[0][tgt[L - 1]]
        outs.append(logZ - gold)
    return np.asarray(outs, np.float32)


# revision 20
# speedup vs baseline: 1.0796x; 1.0796x over previous
"""CRF decoder (logZ - gold) Trainium2 kernel.

Strategy (hardcoded for B=64, S=1024, C=1, N=256, 8 cores):
- Data-parallel over batch: 8 sequences per core.
- Log-semiring forward scan done in *linear* space with a constant host-side
  log-scale sigma = log(256)+0.5 subtracted from each emission, so the scaled
  probabilities p_t stay within fp32/bf16 exponent range for all 1024 steps
  (drift is a mean-zero random walk, ~±3 nats) — no device renormalization.
- Per step: u = W^T p (4 bf16 128x128 matmuls, PSUM fp32), p' = u * E_t (DVE),
  where W = exp(transitions), E_t = exp(em_t - sigma) (ScalarE bulk exp).
- Variable lengths: per-step scalar z_t[b] = p_t . exp(last) via a 1-column
  matmul; host reads z at t = len_b - 1 and assembles
  logZ_b = log z_{len-1} + (len-1)*sigma.  No per-step masking on device.
- Gold emission score on device: one-hot (host-built, masked) times raw
  emissions, multiply+reduce per chunk on DVE, partition-sum on host (tiny).
- Gold transition/head/last scores touch only the tiny parameter tensors and
  targets; computed on host.
"""

import math
from contextlib import ExitStack

import numpy as np
import ml_dtypes

import concourse.bass as bass
import concourse.tile as tile
from concourse import bacc, mybir
from concourse.bass_utils import run_bass_kernel_spmd

B, S, N = 64, 1024, 256
NCORES = 8
BL = B // NCORES  # 8 sequences per core
TC = 128          # time-chunk length
NCHUNK = S // TC
SIGMA = math.log(256.0) + 0.5
ZMIN = 383        # earliest t for which z_t is recorded (lengths >= ZMIN+2 expected)

F32 = mybir.dt.float32
BF16 = mybir.dt.bfloat16


def _crf_tile_kernel(ctx: ExitStack, tc: tile.TileContext, aps: dict,
                     tstars: tuple):
    nc = tc.nc
    em_d, oh_d = aps["em"], aps["oh"]          # [2,128,S,BL] bf16 dram
    w_d = aps["w"]                              # [2,128,2,128] bf16
    el_d = aps["el"]                            # [2,128,1] bf16
    hd_d = aps["hd"]                            # [2,128,1] f32
    zh_d = aps["zhist"]                         # [1, S*BL] f32 out
    ea_d = aps["emitacc"]                       # [128, 2*BL] f32 out

    consts = ctx.enter_context(tc.tile_pool(name="consts", bufs=1))
    state = ctx.enter_context(tc.tile_pool(name="state", bufs=1))
    empool = ctx.enter_context(tc.tile_pool(name="em", bufs=4))
    ohpool = ctx.enter_context(tc.tile_pool(name="oh", bufs=4))
    epool = ctx.enter_context(tc.tile_pool(name="E", bufs=4))
    tmppool = ctx.enter_context(tc.tile_pool(name="tmp", bufs=2))
    redpool = ctx.enter_context(tc.tile_pool(name="red", bufs=2))
    upool = ctx.enter_context(tc.tile_pool(name="u", bufs=6, space="PSUM"))
    zpool = ctx.enter_context(tc.tile_pool(name="z", bufs=2, space="PSUM"))

    # ---- constants into SBUF ----
    w_sb = []   # w_sb[ih][:, jh, :] = W[ih*128:(ih+1)*128, jh*128:(jh+1)*128]
    for ih in range(2):
        t_ = consts.tile([128, 2, 128], BF16, name=f"w{ih}", tag=f"w{ih}")
        nc.sync.dma_start(out=t_[:], in_=w_d[ih])
        w_sb.append(t_)
    el_sb = []
    hd_sb = []
    for ih in range(2):
        e_ = consts.tile([128, 1], BF16, name=f"el{ih}", tag=f"el{ih}")
        nc.sync.dma_start(out=e_[:], in_=el_d[ih])
        el_sb.append(e_)
        h_ = consts.tile([128, 1], F32, name=f"hd{ih}", tag=f"hd{ih}")
        nc.sync.dma_start(out=h_[:], in_=hd_d[ih])
        hd_sb.append(h_)

    sig_sb = consts.tile([128, 1], F32, name="sigb", tag="sigb")
    nc.vector.memset(sig_sb[:], -SIGMA)

    # persistent state: ping-pong p tiles per half
    p_sb = [[state.tile([128, BL], BF16, name=f"p{par}{ih}", tag=f"p{par}{ih}") for ih in range(2)]
            for par in range(2)]
    # z snapshots: one [1,BL] slot per distinct snapshot step (all cores
    # write every slot; host picks its core's column at its length's slot)
    nslots = max(len(tstars), 1)
    zhist = consts.tile([1, nslots * BL], F32, name="zhist", tag="zhist")
    acc = [consts.tile([128, BL], F32, name=f"acc{ih}", tag=f"acc{ih}") for ih in range(2)]
    for ih in range(2):
        nc.vector.memset(acc[ih][:], 0.0)

    alu = mybir.AluOpType

    def do_z(slot, p_pair):
        """z = expLast . p -> zhist[slot]"""
        z = zpool.tile([1, BL], F32, name="z", tag="z")
        nc.tensor.matmul(z[:], el_sb[0][:], p_pair[0][:], start=True, stop=False)
        nc.tensor.matmul(z[:], el_sb[1][:], p_pair[1][:], start=False, stop=True)
        nc.vector.tensor_copy(zhist[:, slot * BL:(slot + 1) * BL], z[:])

    # tstars is the sorted union of distinct snapshot steps; slot = index
    zsteps = {int(t_): k for k, t_ in enumerate(tstars)}

    for c in range(NCHUNK):
        em_sb, oh_sb, e_sb = [], [], []
        for ih in range(2):
            em_t = empool.tile([128, TC, BL], BF16, name=f"emt{ih}", tag=f"em{ih}")
            nc.sync.dma_start(out=em_t[:], in_=em_d[ih, :, c * TC:(c + 1) * TC, :])
            em_sb.append(em_t)
            oh_t = ohpool.tile([128, TC, BL], BF16, name=f"oht{ih}", tag=f"oh{ih}")
            nc.sync.dma_start(out=oh_t[:], in_=oh_d[ih, :, c * TC:(c + 1) * TC, :])
            oh_sb.append(oh_t)
            e_t = epool.tile([128, TC, BL], BF16, name=f"Et{ih}", tag=f"E{ih}")
            nc.scalar.activation(e_t[:], em_t[:],
                                 mybir.ActivationFunctionType.Exp,
                                 bias=sig_sb[:], scale=1.0)
            e_sb.append(e_t)

        if c == 0:
            # p_0 = exp(head + em_0)
            for ih in range(2):
                nc.scalar.activation(p_sb[0][ih][:], em_sb[ih][:, 0, :],
                                     mybir.ActivationFunctionType.Exp,
                                     bias=hd_sb[ih][:], scale=1.0)
            if 0 in zsteps:
                do_z(zsteps[0], p_sb[0])

        # ---- emission gold score for this chunk (DVE, off critical path) ----
        for ih in range(2):
            tmp = tmppool.tile([128, TC, BL], BF16, name=f"tmpt{ih}", tag=f"tmp{ih}")
            nc.vector.tensor_mul(tmp[:], em_sb[ih][:], oh_sb[ih][:])
            red = redpool.tile([128, BL], F32, name=f"redt{ih}", tag=f"red{ih}")
            # reduce over t (innermost after rearrange)
            nc.vector.tensor_reduce(red[:], tmp[:].rearrange("p t b -> p b t"),
                                    mybir.AxisListType.X, alu.add)
            nc.vector.tensor_add(acc[ih][:], acc[ih][:], red[:])

        # ---- the scan steps of this chunk ----
        for r in range(TC):
            t = c * TC + r
            if t == 0:
                continue
            pa = p_sb[(t - 1) % 2]
            pb = p_sb[t % 2]
            u_lo = upool.tile([128, BL], F32, name="u", tag="u")
            u_hi = upool.tile([128, BL], F32, name="u", tag="u")
            # palindrome weight-tile order across steps: even steps
            # W00,W10,W01,W11 — odd steps W11,W01,W10,W00, so the tile at
            # each step boundary repeats (lets HW skip a reload if it can).
            if t % 2 == 1:
                nc.tensor.matmul(u_lo[:], w_sb[0][:, 0, :], pa[0][:], start=True, stop=False)
                nc.tensor.matmul(u_lo[:], w_sb[1][:, 0, :], pa[1][:], start=False, stop=True)
                nc.tensor.matmul(u_hi[:], w_sb[0][:, 1, :], pa[0][:], start=True, stop=False)
                nc.tensor.matmul(u_hi[:], w_sb[1][:, 1, :], pa[1][:], start=False, stop=True)
            else:
                nc.tensor.matmul(u_hi[:], w_sb[1][:, 1, :], pa[1][:], start=True, stop=False)
                nc.tensor.matmul(u_hi[:], w_sb[0][:, 1, :], pa[0][:], start=False, stop=True)
                nc.tensor.matmul(u_lo[:], w_sb[1][:, 0, :], pa[1][:], start=True, stop=False)
                nc.tensor.matmul(u_lo[:], w_sb[0][:, 0, :], pa[0][:], start=False, stop=True)
            nc.vector.tensor_mul(pb[0][:], u_lo[:], e_sb[0][:, r, :])
            nc.vector.tensor_mul(pb[1][:], u_hi[:], e_sb[1][:, r, :])
            if t in zsteps:
                do_z(zsteps[t], pb)

    # ---- outputs ----
    nc.sync.dma_start(out=zh_d[:], in_=zhist[:])
    nc.sync.dma_start(out=ea_d[:, 0:BL], in_=acc[0][:])
    nc.sync.dma_start(out=ea_d[:, BL:2 * BL], in_=acc[1][:])


_NC_CACHE = {}


def _build_nc(tstars=(S - 1,)):
    """tstars: sorted union (over all cores/sequences) of snapshot steps
    len_b - 1. SPMD — the single shared program snapshots z at every such
    step into its own slot; each core's host-side assembly picks its column.
    """
    key = tuple(tstars)
    if key in _NC_CACHE:
        return _NC_CACHE[key]
    nc = bacc.Bacc("TRN2", target_bir_lowering=False, debug=False,
                   num_devices=NCORES)
    aps = {
        "em": nc.dram_tensor("em", [2, 128, S, BL], BF16, kind="ExternalInput").ap(),
        "oh": nc.dram_tensor("oh", [2, 128, S, BL], BF16, kind="ExternalInput").ap(),
        "w": nc.dram_tensor("w", [2, 128, 2, 128], BF16, kind="ExternalInput").ap(),
        "el": nc.dram_tensor("el", [2, 128, 1], BF16, kind="ExternalInput").ap(),
        "hd": nc.dram_tensor("hd", [2, 128, 1], F32, kind="ExternalInput").ap(),
        "zhist": nc.dram_tensor("zhist", [1, max(len(tstars), 1) * BL], F32,
                                kind="ExternalOutput").ap(),
        "emitacc": nc.dram_tensor("emitacc", [128, 2 * BL], F32, kind="ExternalOutput").ap(),
    }
    with tile.TileContext(nc) as tc:
        with ExitStack() as ctx:
            _crf_tile_kernel(ctx, tc, aps, tuple(tstars))
    nc.compile()
    _NC_CACHE[key] = nc
    return nc


def _host_gold_small(targets, lengths, transitions, head_transitions, last_transitions):
    """Transition/head/last parts of the gold score (no big-tensor access)."""
    T = transitions[0].astype(np.float64)
    tr = T[targets[:, :-1], targets[:, 1:]]                       # [B,S-1]
    pmask = (np.arange(1, S)[None, :] < lengths[:, None])
    trans_score = (tr * pmask).sum(1)
    head_score = head_transitions[0][targets[:, 0]].astype(np.float64)
    last_tag = np.take_along_axis(targets, (lengths - 1)[:, None], axis=1)[:, 0]
    last_score = last_transitions[0][last_tag].astype(np.float64)
    return trans_score + head_score + last_score


def _make_in_maps(inputs):
    emissions = np.asarray(inputs["emissions"])
    targets = np.asarray(inputs["targets"])
    lengths = np.asarray(inputs["lengths"])
    transitions = np.asarray(inputs["transitions"])
    head_transitions = np.asarray(inputs["head_transitions"])
    last_transitions = np.asarray(inputs["last_transitions"])

    W = np.exp(transitions[0].astype(np.float64)).astype(ml_dtypes.bfloat16)
    w_sh = np.ascontiguousarray(W.reshape(2, 128, 2, 128))
    el_sh = np.ascontiguousarray(
        np.exp(last_transitions[0].astype(np.float64))
        .astype(ml_dtypes.bfloat16).reshape(2, 128, 1))
    hd_sh = np.ascontiguousarray(
        head_transitions[0].astype(np.float32).reshape(2, 128, 1))

    em_bf = emissions[:, :, 0, :].astype(ml_dtypes.bfloat16)      # [B,S,N]

    in_maps = []
    for c in range(NCORES):
        sl = slice(c * BL, (c + 1) * BL)
        em_c = np.ascontiguousarray(
            em_bf[sl].transpose(2, 1, 0).reshape(2, 128, S, BL))  # [j,t,b]
        tgt_c = targets[sl]                                       # [BL,S]
        len_c = lengths[sl]
        oh_c = np.zeros((N, S, BL), dtype=ml_dtypes.bfloat16)
        bb, tt = np.meshgrid(np.arange(BL), np.arange(S), indexing="ij")
        valid = tt < len_c[:, None]
        oh_c[tgt_c[bb[valid], tt[valid]], tt[valid], bb[valid]] = 1.0
        oh_c = np.ascontiguousarray(oh_c.reshape(2, 128, S, BL))
        in_maps.append({"em": em_c, "oh": oh_c, "w": w_sh, "el": el_sh,
                        "hd": hd_sh})
    return in_maps


def kernel(emissions, targets, lengths, transitions, head_transitions,
           last_transitions):
    emissions = np.asarray(emissions)
    targets = np.asarray(targets)
    lengths = np.asarray(lengths)
    transitions = np.asarray(transitions)
    head_transitions = np.asarray(head_transitions)
    last_transitions = np.asarray(last_transitions)
    assert emissions.shape == (B, S, 1, N), emissions.shape

    tstar = np.clip(lengths - 1, 0, S - 1)
    tstars = tuple(sorted(set(int(t) for t in tstar)))
    nc = _build_nc(tstars)
    slot_of = {t: k for k, t in enumerate(tstars)}
    in_maps = _make_in_maps(dict(
        emissions=emissions, targets=targets, lengths=lengths,
        transitions=transitions, head_transitions=head_transitions,
        last_transitions=last_transitions))

    res = run_bass_kernel_spmd(nc, in_maps, list(range(NCORES)))

    logZ = np.zeros(B, np.float64)
    emit = np.zeros(B, np.float64)
    for c in range(NCORES):
        zh = res.results[c]["zhist"].reshape(len(tstars), BL).astype(np.float64)
        ea = res.results[c]["emitacc"].astype(np.float64)         # [128, 2*BL]
        for bl in range(BL):
            b = c * BL + bl
            logZ[b] = np.log(zh[slot_of[int(tstar[b])], bl]) + tstar[b] * SIGMA
            emit[b] = ea[:, bl].sum() + ea[:, BL + bl].sum()

    gold = emit + _host_gold_small(targets, lengths, transitions,
                                   head_transitions, last_transitions)
    return (logZ - gold).astype(np.float32)[:, None]              # [B, C=1]


def _host_exact(emissions, targets, lengths, transitions, head_transitions,
                last_transitions, idx):
    from scipy.special import logsumexp  # noqa — tiny fallback, rarely used
    outs = []
    T = transitions[0].astype(np.float64)
    for b in idx:
        L = int(lengths[b])
        em = emissions[b, :, 0, :].astype(np.float64)
        alpha = head_transitions[0].astype(np.float64) + em[0]
        for t in range(1, L):
            alpha = logsumexp(alpha[:, None] + T, axis=0) + em[t]
        logZ = logsumexp(alpha + last_transitions[0].astype(np.float64))
        tgt = targets[b]
        gold = em[np.arange(L), tgt[:L]].sum()
        gold += T[tgt[:L - 1], tgt[1:L]].sum()
        gold += head_transitions[0][tgt[0]] + last_transitions[0][tgt[L - 1]]
        outs.append(logZ - gold)
    return np.asarray(outs, np.float32)


# revision 21
# speedup vs baseline: 1.0809x; 1.0013x over previous
"""CRF decoder (logZ - gold) Trainium2 kernel.

Strategy (hardcoded for B=64, S=1024, C=1, N=256, 8 cores):
- Data-parallel over batch: 8 sequences per core.
- Log-semiring forward scan done in *linear* space with a constant host-side
  log-scale sigma = log(256)+0.5 subtracted from each emission, so the scaled
  probabilities p_t stay within fp32/bf16 exponent range for all 1024 steps
  (drift is a mean-zero random walk, ~±3 nats) — no device renormalization.
- Per step: u = W^T p (4 bf16 128x128 matmuls, PSUM fp32), p' = u * E_t (DVE),
  where W = exp(transitions), E_t = exp(em_t - sigma) (ScalarE bulk exp).
- Variable lengths: per-step scalar z_t[b] = p_t . exp(last) via a 1-column
  matmul; host reads z at t = len_b - 1 and assembles
  logZ_b = log z_{len-1} + (len-1)*sigma.  No per-step masking on device.
- Gold emission score on device: one-hot (host-built, masked) times raw
  emissions, multiply+reduce per chunk on DVE, partition-sum on host (tiny).
- Gold transition/head/last scores touch only the tiny parameter tensors and
  targets; computed on host.
"""

import math
from contextlib import ExitStack

import numpy as np
import ml_dtypes

import concourse.bass as bass
import concourse.tile as tile
from concourse import bacc, mybir
from concourse.bass_utils import run_bass_kernel_spmd

B, S, N = 64, 1024, 256
NCORES = 8
BL = B // NCORES  # 8 sequences per core
TC = 128          # time-chunk length
NCHUNK = S // TC
SIGMA = math.log(256.0) + 0.5
ZMIN = 383        # earliest t for which z_t is recorded (lengths >= ZMIN+2 expected)

F32 = mybir.dt.float32
BF16 = mybir.dt.bfloat16


def _crf_tile_kernel(ctx: ExitStack, tc: tile.TileContext, aps: dict,
                     tstars: tuple):
    nc = tc.nc
    em_d, oh_d = aps["em"], aps["oh"]          # [2,128,S,BL] bf16 dram
    w_d = aps["w"]                              # [2,128,2,128] bf16
    el_d = aps["el"]                            # [2,128,1] bf16
    hd_d = aps["hd"]                            # [2,128,1] f32
    zh_d = aps["zhist"]                         # [1, S*BL] f32 out
    ea_d = aps["emitacc"]                       # [128, 2*BL] f32 out

    consts = ctx.enter_context(tc.tile_pool(name="consts", bufs=1))
    state = ctx.enter_context(tc.tile_pool(name="state", bufs=1))
    empool = ctx.enter_context(tc.tile_pool(name="em", bufs=4))
    ohpool = ctx.enter_context(tc.tile_pool(name="oh", bufs=4))
    epool = ctx.enter_context(tc.tile_pool(name="E", bufs=4))
    tmppool = ctx.enter_context(tc.tile_pool(name="tmp", bufs=2))
    redpool = ctx.enter_context(tc.tile_pool(name="red", bufs=2))
    upool = ctx.enter_context(tc.tile_pool(name="u", bufs=6, space="PSUM"))
    zpool = ctx.enter_context(tc.tile_pool(name="z", bufs=2, space="PSUM"))

    # ---- constants into SBUF ----
    w_sb = []   # w_sb[ih][:, jh, :] = W[ih*128:(ih+1)*128, jh*128:(jh+1)*128]
    for ih in range(2):
        t_ = consts.tile([128, 2, 128], BF16, name=f"w{ih}", tag=f"w{ih}")
        nc.sync.dma_start(out=t_[:], in_=w_d[ih])
        w_sb.append(t_)
    el_sb = []
    hd_sb = []
    for ih in range(2):
        e_ = consts.tile([128, 1], BF16, name=f"el{ih}", tag=f"el{ih}")
        nc.sync.dma_start(out=e_[:], in_=el_d[ih])
        el_sb.append(e_)
        h_ = consts.tile([128, 1], F32, name=f"hd{ih}", tag=f"hd{ih}")
        nc.sync.dma_start(out=h_[:], in_=hd_d[ih])
        hd_sb.append(h_)

    sig_sb = consts.tile([128, 1], F32, name="sigb", tag="sigb")
    nc.vector.memset(sig_sb[:], -SIGMA)

    # persistent state: ping-pong p tiles [128, (jh, b)]
    p_sb = [state.tile([128, 2, BL], BF16, name=f"p{par}", tag=f"p{par}")
            for par in range(2)]
    # z snapshots: one [1,BL] slot per distinct snapshot step (all cores
    # write every slot; host picks its core's column at its length's slot)
    nslots = max(len(tstars), 1)
    zhist = consts.tile([1, nslots * BL], F32, name="zhist", tag="zhist")
    acc = consts.tile([128, 2, BL], F32, name="acc", tag="acc")
    nc.vector.memset(acc[:], 0.0)

    alu = mybir.AluOpType

    def do_z(slot, p_pair):
        """z = expLast . p -> zhist[slot]"""
        z = zpool.tile([1, BL], F32, name="z", tag="z")
        nc.tensor.matmul(z[:], el_sb[0][:], p_pair[:, 0, :], start=True, stop=False)
        nc.tensor.matmul(z[:], el_sb[1][:], p_pair[:, 1, :], start=False, stop=True)
        nc.vector.tensor_copy(zhist[:, slot * BL:(slot + 1) * BL], z[:])

    # tstars is the sorted union of distinct snapshot steps; slot = index
    zsteps = {int(t_): k for k, t_ in enumerate(tstars)}

    for c in range(NCHUNK):
        em_t = empool.tile([128, TC, 2, BL], BF16, name="emt", tag="em")
        nc.sync.dma_start(out=em_t[:], in_=em_d[:, c * TC:(c + 1) * TC, :, :])
        oh_t = ohpool.tile([128, TC, 2, BL], BF16, name="oht", tag="oh")
        nc.sync.dma_start(out=oh_t[:], in_=oh_d[:, c * TC:(c + 1) * TC, :, :])
        e_t = epool.tile([128, TC, 2, BL], BF16, name="Et", tag="E")
        nc.scalar.activation(e_t[:], em_t[:],
                             mybir.ActivationFunctionType.Exp,
                             bias=sig_sb[:], scale=1.0)

        if c == 0:
            # p_0 = exp(head + em_0)
            for ih in range(2):
                nc.scalar.activation(p_sb[0][:, ih, :], em_t[:, 0, ih, :],
                                     mybir.ActivationFunctionType.Exp,
                                     bias=hd_sb[ih][:], scale=1.0)
            if 0 in zsteps:
                do_z(zsteps[0], p_sb[0])

        # ---- emission gold score for this chunk (DVE, off critical path) ----
        tmp = tmppool.tile([128, TC, 2, BL], BF16, name="tmpt", tag="tmp")
        nc.vector.tensor_mul(tmp[:], em_t[:], oh_t[:])
        red = redpool.tile([128, 2, BL], F32, name="redt", tag="red")
        nc.vector.tensor_reduce(red[:], tmp[:].rearrange("p t h b -> p h b t"),
                                mybir.AxisListType.X, alu.add)
        nc.vector.tensor_add(acc[:], acc[:], red[:])

        # ---- the scan steps of this chunk ----
        for r in range(TC):
            t = c * TC + r
            if t == 0:
                continue
            pa = p_sb[(t - 1) % 2]
            pb = p_sb[t % 2]
            u = upool.tile([128, 2, BL], F32, name="u", tag="u")
            # palindrome weight-tile order across steps so the boundary tile
            # repeats (lets HW skip a reload if it can)
            if t % 2 == 1:
                nc.tensor.matmul(u[:, 0, :], w_sb[0][:, 0, :], pa[:, 0, :], start=True, stop=False)
                nc.tensor.matmul(u[:, 0, :], w_sb[1][:, 0, :], pa[:, 1, :], start=False, stop=True)
                nc.tensor.matmul(u[:, 1, :], w_sb[0][:, 1, :], pa[:, 0, :], start=True, stop=False)
                nc.tensor.matmul(u[:, 1, :], w_sb[1][:, 1, :], pa[:, 1, :], start=False, stop=True)
            else:
                nc.tensor.matmul(u[:, 1, :], w_sb[1][:, 1, :], pa[:, 1, :], start=True, stop=False)
                nc.tensor.matmul(u[:, 1, :], w_sb[0][:, 1, :], pa[:, 0, :], start=False, stop=True)
                nc.tensor.matmul(u[:, 0, :], w_sb[1][:, 0, :], pa[:, 1, :], start=True, stop=False)
                nc.tensor.matmul(u[:, 0, :], w_sb[0][:, 0, :], pa[:, 0, :], start=False, stop=True)
            nc.vector.tensor_mul(pb[:], u[:], e_t[:, r, :, :])
            if t in zsteps:
                do_z(zsteps[t], pb)

    # ---- outputs ----
    nc.sync.dma_start(out=zh_d[:], in_=zhist[:])
    nc.sync.dma_start(out=ea_d[:], in_=acc[:].rearrange("p h b -> p (h b)"))


_NC_CACHE = {}


def _build_nc(tstars=(S - 1,)):
    """tstars: sorted union (over all cores/sequences) of snapshot steps
    len_b - 1. SPMD — the single shared program snapshots z at every such
    step into its own slot; each core's host-side assembly picks its column.
    """
    key = tuple(tstars)
    if key in _NC_CACHE:
        return _NC_CACHE[key]
    nc = bacc.Bacc("TRN2", target_bir_lowering=False, debug=False,
                   num_devices=NCORES)
    aps = {
        "em": nc.dram_tensor("em", [128, S, 2, BL], BF16, kind="ExternalInput").ap(),
        "oh": nc.dram_tensor("oh", [128, S, 2, BL], BF16, kind="ExternalInput").ap(),
        "w": nc.dram_tensor("w", [2, 128, 2, 128], BF16, kind="ExternalInput").ap(),
        "el": nc.dram_tensor("el", [2, 128, 1], BF16, kind="ExternalInput").ap(),
        "hd": nc.dram_tensor("hd", [2, 128, 1], F32, kind="ExternalInput").ap(),
        "zhist": nc.dram_tensor("zhist", [1, max(len(tstars), 1) * BL], F32,
                                kind="ExternalOutput").ap(),
        "emitacc": nc.dram_tensor("emitacc", [128, 2 * BL], F32, kind="ExternalOutput").ap(),
    }
    with tile.TileContext(nc) as tc:
        with ExitStack() as ctx:
            _crf_tile_kernel(ctx, tc, aps, tuple(tstars))
    nc.compile()
    _NC_CACHE[key] = nc
    return nc


def _host_gold_small(targets, lengths, transitions, head_transitions, last_transitions):
    """Transition/head/last parts of the gold score (no big-tensor access)."""
    T = transitions[0].astype(np.float64)
    tr = T[targets[:, :-1], targets[:, 1:]]                       # [B,S-1]
    pmask = (np.arange(1, S)[None, :] < lengths[:, None])
    trans_score = (tr * pmask).sum(1)
    head_score = head_transitions[0][targets[:, 0]].astype(np.float64)
    last_tag = np.take_along_axis(targets, (lengths - 1)[:, None], axis=1)[:, 0]
    last_score = last_transitions[0][last_tag].astype(np.float64)
    return trans_score + head_score + last_score


def _make_in_maps(inputs):
    emissions = np.asarray(inputs["emissions"])
    targets = np.asarray(inputs["targets"])
    lengths = np.asarray(inputs["lengths"])
    transitions = np.asarray(inputs["transitions"])
    head_transitions = np.asarray(inputs["head_transitions"])
    last_transitions = np.asarray(inputs["last_transitions"])

    W = np.exp(transitions[0].astype(np.float64)).astype(ml_dtypes.bfloat16)
    w_sh = np.ascontiguousarray(W.reshape(2, 128, 2, 128))
    el_sh = np.ascontiguousarray(
        np.exp(last_transitions[0].astype(np.float64))
        .astype(ml_dtypes.bfloat16).reshape(2, 128, 1))
    hd_sh = np.ascontiguousarray(
        head_transitions[0].astype(np.float32).reshape(2, 128, 1))

    em_bf = emissions[:, :, 0, :].astype(ml_dtypes.bfloat16)      # [B,S,N]

    in_maps = []
    for c in range(NCORES):
        sl = slice(c * BL, (c + 1) * BL)
        em_c = np.ascontiguousarray(
            em_bf[sl].transpose(2, 1, 0).reshape(2, 128, S, BL)
            .transpose(1, 2, 0, 3))                   # [jlo, t, jh, b]
        tgt_c = targets[sl]                                       # [BL,S]
        len_c = lengths[sl]
        oh_c = np.zeros((N, S, BL), dtype=ml_dtypes.bfloat16)
        bb, tt = np.meshgrid(np.arange(BL), np.arange(S), indexing="ij")
        valid = tt < len_c[:, None]
        oh_c[tgt_c[bb[valid], tt[valid]], tt[valid], bb[valid]] = 1.0
        oh_c = np.ascontiguousarray(
            oh_c.reshape(2, 128, S, BL).transpose(1, 2, 0, 3))
        in_maps.append({"em": em_c, "oh": oh_c, "w": w_sh, "el": el_sh,
                        "hd": hd_sh})
    return in_maps


def kernel(emissions, targets, lengths, transitions, head_transitions,
           last_transitions):
    emissions = np.asarray(emissions)
    targets = np.asarray(targets)
    lengths = np.asarray(lengths)
    transitions = np.asarray(transitions)
    head_transitions = np.asarray(head_transitions)
    last_transitions = np.asarray(last_transitions)
    assert emissions.shape == (B, S, 1, N), emissions.shape

    tstar = np.clip(lengths - 1, 0, S - 1)
    tstars = tuple(sorted(set(int(t) for t in tstar)))
    nc = _build_nc(tstars)
    slot_of = {t: k for k, t in enumerate(tstars)}
    in_maps = _make_in_maps(dict(
        emissions=emissions, targets=targets, lengths=lengths,
        transitions=transitions, head_transitions=head_transitions,
        last_transitions=last_transitions))

    res = run_bass_kernel_spmd(nc, in_maps, list(range(NCORES)))

    logZ = np.zeros(B, np.float64)
    emit = np.zeros(B, np.float64)
    for c in range(NCORES):
        zh = res.results[c]["zhist"].reshape(len(tstars), BL).astype(np.float64)
        ea = res.results[c]["emitacc"].astype(np.float64)         # [128, 2*BL]
        for bl in range(BL):
            b = c * BL + bl
            logZ[b] = np.log(zh[slot_of[int(tstar[b])], bl]) + tstar[b] * SIGMA
            emit[b] = ea[:, bl].sum() + ea[:, BL + bl].sum()

    gold = emit + _host_gold_small(targets, lengths, transitions,
                                   head_transitions, last_transitions)
    return (logZ - gold).astype(np.float32)[:, None]              # [B, C=1]


def _host_exact(emissions, targets, lengths, transitions, head_transitions,
                last_transitions, idx):
    from scipy.special import logsumexp  # noqa — tiny fallback, rarely used
    outs = []
    T = transitions[0].astype(np.float64)
    for b in idx:
        L = int(lengths[b])
        em = emissions[b, :, 0, :].astype(np.float64)
        alpha = head_transitions[0].astype(np.float64) + em[0]
        for t in range(1, L):
            alpha = logsumexp(alpha[:, None] + T, axis=0) + em[t]
        logZ = logsumexp(alpha + last_transitions[0].astype(np.float64))
        tgt = targets[b]
        gold = em[np.arange(L), tgt[:L]].sum()
        gold += T[tgt[:L - 1], tgt[1:L]].sum()
        gold += head_transitions[0][tgt[0]] + last_transitions[0][tgt[L - 1]]
        outs.append(logZ - gold)
    return np.asarray(outs, np.float32)


# revision 23
# speedup vs baseline: 1.0960x; 1.0139x over previous
"""CRF decoder (logZ - gold) Trainium2 kernel.

Strategy (hardcoded for B=64, S=1024, C=1, N=256, 8 cores):
- Data-parallel over batch: 8 sequences per core.
- Log-semiring forward scan done in *linear* space with a constant host-side
  log-scale sigma = log(256)+0.5 subtracted from each emission, so the scaled
  probabilities p_t stay within fp32/bf16 exponent range for all 1024 steps
  (drift is a mean-zero random walk, ~±3 nats) — no device renormalization.
- Per step: u = W^T p (4 bf16 128x128 matmuls, PSUM fp32), p' = u * E_t (DVE),
  where W = exp(transitions), E_t = exp(em_t - sigma) (ScalarE bulk exp).
- Variable lengths: per-step scalar z_t[b] = p_t . exp(last) via a 1-column
  matmul; host reads z at t = len_b - 1 and assembles
  logZ_b = log z_{len-1} + (len-1)*sigma.  No per-step masking on device.
- Gold emission score on device: one-hot (host-built, masked) times raw
  emissions, multiply+reduce per chunk on DVE, partition-sum on host (tiny).
- Gold transition/head/last scores touch only the tiny parameter tensors and
  targets; computed on host.
"""

import math
from contextlib import ExitStack

import numpy as np
import ml_dtypes

import concourse.bass as bass
import concourse.tile as tile
from concourse import bacc, mybir
from concourse.bass_utils import run_bass_kernel_spmd

B, S, N = 64, 1024, 256
NCORES = 8
BL = B // NCORES  # 8 sequences per core
TC = 128          # time-chunk length
NCHUNK = S // TC
SIGMA = math.log(256.0) + 0.5
ZMIN = 383        # earliest t for which z_t is recorded (lengths >= ZMIN+2 expected)

F32 = mybir.dt.float32
BF16 = mybir.dt.bfloat16


def _crf_tile_kernel(ctx: ExitStack, tc: tile.TileContext, aps: dict,
                     tstars: tuple):
    nc = tc.nc
    em_d, oh_d = aps["em"], aps["oh"]          # [2,128,S,BL] bf16 dram
    w_d = aps["w"]                              # [2,128,2,128] bf16
    el_d = aps["el"]                            # [2,128,1] bf16
    hd_d = aps["hd"]                            # [2,128,1] f32
    zh_d = aps["zhist"]                         # [1, S*BL] f32 out
    ea_d = aps["emitacc"]                       # [128, 2*BL] f32 out

    consts = ctx.enter_context(tc.tile_pool(name="consts", bufs=1))
    state = ctx.enter_context(tc.tile_pool(name="state", bufs=1))
    empool = ctx.enter_context(tc.tile_pool(name="em", bufs=4))
    ohpool = ctx.enter_context(tc.tile_pool(name="oh", bufs=4))
    epool = ctx.enter_context(tc.tile_pool(name="E", bufs=4))
    tmppool = ctx.enter_context(tc.tile_pool(name="tmp", bufs=2))
    redpool = ctx.enter_context(tc.tile_pool(name="red", bufs=2))
    upool = ctx.enter_context(tc.tile_pool(name="u", bufs=3, space="PSUM"))
    zpool = ctx.enter_context(tc.tile_pool(name="z", bufs=2, space="PSUM"))

    # ---- constants into SBUF ----
    w_sb = []   # w_sb[ih][:, jh, :] = W[ih*128:(ih+1)*128, jh*128:(jh+1)*128]
    for ih in range(2):
        t_ = consts.tile([128, 2, 128], BF16, name=f"w{ih}", tag=f"w{ih}")
        nc.sync.dma_start(out=t_[:], in_=w_d[ih])
        w_sb.append(t_)
    el_sb = []
    hd_sb = []
    for ih in range(2):
        e_ = consts.tile([128, 1], BF16, name=f"el{ih}", tag=f"el{ih}")
        nc.sync.dma_start(out=e_[:], in_=el_d[ih])
        el_sb.append(e_)
        h_ = consts.tile([128, 1], F32, name=f"hd{ih}", tag=f"hd{ih}")
        nc.sync.dma_start(out=h_[:], in_=hd_d[ih])
        hd_sb.append(h_)

    sig_sb = consts.tile([128, 1], F32, name="sigb", tag="sigb")
    nc.vector.memset(sig_sb[:], -SIGMA)

    # persistent state: ping-pong p tiles [128, (jh, b)], per group
    GB = BL // 2
    p_sb = [[state.tile([128, 2, GB], BF16, name=f"p{par}g{g}", tag=f"p{par}g{g}")
             for g in range(2)] for par in range(2)]
    # z snapshots: one [1,BL] slot per distinct snapshot step (all cores
    # write every slot; host picks its core's column at its length's slot)
    nslots = max(len(tstars), 1)
    zhist = consts.tile([1, nslots * BL], F32, name="zhist", tag="zhist")
    acc = consts.tile([128, 2, BL], F32, name="acc", tag="acc")
    nc.vector.memset(acc[:], 0.0)

    alu = mybir.AluOpType

    def do_z(slot, p_pair):
        """z = expLast . p -> zhist[slot] (both groups)"""
        for g in range(2):
            z = zpool.tile([1, GB], F32, name="z", tag="z")
            nc.tensor.matmul(z[:], el_sb[0][:], p_pair[g][:, 0, :], start=True, stop=False)
            nc.tensor.matmul(z[:], el_sb[1][:], p_pair[g][:, 1, :], start=False, stop=True)
            nc.vector.tensor_copy(
                zhist[:, slot * BL + g * GB: slot * BL + (g + 1) * GB], z[:])

    # tstars is the sorted union of distinct snapshot steps; slot = index
    zsteps = {int(t_): k for k, t_ in enumerate(tstars)}

    for c in range(NCHUNK):
        em_t = empool.tile([128, TC, 2, BL], BF16, name="emt", tag="em")
        nc.sync.dma_start(out=em_t[:], in_=em_d[:, c * TC:(c + 1) * TC, :, :])
        oh_t = ohpool.tile([128, TC, 2, BL], BF16, name="oht", tag="oh")
        nc.sync.dma_start(out=oh_t[:], in_=oh_d[:, c * TC:(c + 1) * TC, :, :])
        e_t = epool.tile([128, TC, 2, BL], BF16, name="Et", tag="E")
        nc.scalar.activation(e_t[:], em_t[:],
                             mybir.ActivationFunctionType.Exp,
                             bias=sig_sb[:], scale=1.0)

        if c == 0:
            # p_0 = exp(head + em_0)
            for g in range(2):
                for ih in range(2):
                    nc.scalar.activation(
                        p_sb[0][g][:, ih, :],
                        em_t[:, 0, ih, g * GB:(g + 1) * GB],
                        mybir.ActivationFunctionType.Exp,
                        bias=hd_sb[ih][:], scale=1.0)
            if 0 in zsteps:
                do_z(zsteps[0], p_sb[0])

        # ---- emission gold score for this chunk (DVE, off critical path) ----
        tmp = tmppool.tile([128, TC, 2, BL], BF16, name="tmpt", tag="tmp")
        nc.vector.tensor_mul(tmp[:], em_t[:], oh_t[:])
        red = redpool.tile([128, 2, BL], F32, name="redt", tag="red")
        nc.vector.tensor_reduce(red[:], tmp[:].rearrange("p t h b -> p h b t"),
                                mybir.AxisListType.X, alu.add)
        nc.vector.tensor_add(acc[:], acc[:], red[:])

        # ---- the scan steps of this chunk ----
        for r in range(TC):
            t = c * TC + r
            if t == 0:
                continue
            pa = p_sb[(t - 1) % 2]
            pb = p_sb[t % 2]
            # two independent sequence groups: group B's matmuls fill the
            # PE while group A's vector multiply + sync latency elapse.
            # Weight-tile orders arranged so consecutive bursts start with
            # the tile the previous burst ended on (A fwd, B reversed).
            us = []
            for g in range(2):
                u = upool.tile([128, 2, GB], F32, name=f"u{g}", tag=f"u{g}")
                us.append(u)
                p_ = pa[g]
                if g == 0:
                    nc.tensor.matmul(u[:, 0, :], w_sb[0][:, 0, :], p_[:, 0, :], start=True, stop=False)
                    nc.tensor.matmul(u[:, 0, :], w_sb[1][:, 0, :], p_[:, 1, :], start=False, stop=True)
                    nc.tensor.matmul(u[:, 1, :], w_sb[0][:, 1, :], p_[:, 0, :], start=True, stop=False)
                    nc.tensor.matmul(u[:, 1, :], w_sb[1][:, 1, :], p_[:, 1, :], start=False, stop=True)
                else:
                    nc.tensor.matmul(u[:, 1, :], w_sb[1][:, 1, :], p_[:, 1, :], start=True, stop=False)
                    nc.tensor.matmul(u[:, 1, :], w_sb[0][:, 1, :], p_[:, 0, :], start=False, stop=True)
                    nc.tensor.matmul(u[:, 0, :], w_sb[1][:, 0, :], p_[:, 1, :], start=True, stop=False)
                    nc.tensor.matmul(u[:, 0, :], w_sb[0][:, 0, :], p_[:, 0, :], start=False, stop=True)
            for g in range(2):
                nc.vector.tensor_mul(pb[g][:], us[g][:],
                                     e_t[:, r, :, g * GB:(g + 1) * GB])
            if t in zsteps:
                do_z(zsteps[t], pb)

    # ---- outputs ----
    nc.sync.dma_start(out=zh_d[:], in_=zhist[:])
    nc.sync.dma_start(out=ea_d[:], in_=acc[:].rearrange("p h b -> p (h b)"))


_NC_CACHE = {}


def _build_nc(tstars=(S - 1,)):
    """tstars: sorted union (over all cores/sequences) of snapshot steps
    len_b - 1. SPMD — the single shared program snapshots z at every such
    step into its own slot; each core's host-side assembly picks its column.
    """
    key = tuple(tstars)
    if key in _NC_CACHE:
        return _NC_CACHE[key]
    nc = bacc.Bacc("TRN2", target_bir_lowering=False, debug=False,
                   num_devices=NCORES)
    aps = {
        "em": nc.dram_tensor("em", [128, S, 2, BL], BF16, kind="ExternalInput").ap(),
        "oh": nc.dram_tensor("oh", [128, S, 2, BL], BF16, kind="ExternalInput").ap(),
        "w": nc.dram_tensor("w", [2, 128, 2, 128], BF16, kind="ExternalInput").ap(),
        "el": nc.dram_tensor("el", [2, 128, 1], BF16, kind="ExternalInput").ap(),
        "hd": nc.dram_tensor("hd", [2, 128, 1], F32, kind="ExternalInput").ap(),
        "zhist": nc.dram_tensor("zhist", [1, max(len(tstars), 1) * BL], F32,
                                kind="ExternalOutput").ap(),
        "emitacc": nc.dram_tensor("emitacc", [128, 2 * BL], F32, kind="ExternalOutput").ap(),
    }
    with tile.TileContext(nc) as tc:
        with ExitStack() as ctx:
            _crf_tile_kernel(ctx, tc, aps, tuple(tstars))
    nc.compile()
    _NC_CACHE[key] = nc
    return nc


def _host_gold_small(targets, lengths, transitions, head_transitions, last_transitions):
    """Transition/head/last parts of the gold score (no big-tensor access)."""
    T = transitions[0].astype(np.float64)
    tr = T[targets[:, :-1], targets[:, 1:]]                       # [B,S-1]
    pmask = (np.arange(1, S)[None, :] < lengths[:, None])
    trans_score = (tr * pmask).sum(1)
    head_score = head_transitions[0][targets[:, 0]].astype(np.float64)
    last_tag = np.take_along_axis(targets, (lengths - 1)[:, None], axis=1)[:, 0]
    last_score = last_transitions[0][last_tag].astype(np.float64)
    return trans_score + head_score + last_score


def _make_in_maps(inputs):
    emissions = np.asarray(inputs["emissions"])
    targets = np.asarray(inputs["targets"])
    lengths = np.asarray(inputs["lengths"])
    transitions = np.asarray(inputs["transitions"])
    head_transitions = np.asarray(inputs["head_transitions"])
    last_transitions = np.asarray(inputs["last_transitions"])

    W = np.exp(transitions[0].astype(np.float64)).astype(ml_dtypes.bfloat16)
    w_sh = np.ascontiguousarray(W.reshape(2, 128, 2, 128))
    el_sh = np.ascontiguousarray(
        np.exp(last_transitions[0].astype(np.float64))
        .astype(ml_dtypes.bfloat16).reshape(2, 128, 1))
    hd_sh = np.ascontiguousarray(
        head_transitions[0].astype(np.float32).reshape(2, 128, 1))

    em_bf = emissions[:, :, 0, :].astype(ml_dtypes.bfloat16)      # [B,S,N]

    in_maps = []
    for c in range(NCORES):
        sl = slice(c * BL, (c + 1) * BL)
        em_c = np.ascontiguousarray(
            em_bf[sl].transpose(2, 1, 0).reshape(2, 128, S, BL)
            .transpose(1, 2, 0, 3))                   # [jlo, t, jh, b]
        tgt_c = targets[sl]                                       # [BL,S]
        len_c = lengths[sl]
        oh_c = np.zeros((N, S, BL), dtype=ml_dtypes.bfloat16)
        bb, tt = np.meshgrid(np.arange(BL), np.arange(S), indexing="ij")
        valid = tt < len_c[:, None]
        oh_c[tgt_c[bb[valid], tt[valid]], tt[valid], bb[valid]] = 1.0
        oh_c = np.ascontiguousarray(
            oh_c.reshape(2, 128, S, BL).transpose(1, 2, 0, 3))
        in_maps.append({"em": em_c, "oh": oh_c, "w": w_sh, "el": el_sh,
                        "hd": hd_sh})
    return in_maps


def kernel(emissions, targets, lengths, transitions, head_transitions,
           last_transitions):
    emissions = np.asarray(emissions)
    targets = np.asarray(targets)
    lengths = np.asarray(lengths)
    transitions = np.asarray(transitions)
    head_transitions = np.asarray(head_transitions)
    last_transitions = np.asarray(last_transitions)
    assert emissions.shape == (B, S, 1, N), emissions.shape

    tstar = np.clip(lengths - 1, 0, S - 1)
    tstars = tuple(sorted(set(int(t) for t in tstar)))
    nc = _build_nc(tstars)
    slot_of = {t: k for k, t in enumerate(tstars)}
    in_maps = _make_in_maps(dict(
        emissions=emissions, targets=targets, lengths=lengths,
        transitions=transitions, head_transitions=head_transitions,
        last_transitions=last_transitions))

    res = run_bass_kernel_spmd(nc, in_maps, list(range(NCORES)))

    logZ = np.zeros(B, np.float64)
    emit = np.zeros(B, np.float64)
    for c in range(NCORES):
        zh = res.results[c]["zhist"].reshape(len(tstars), BL).astype(np.float64)
        ea = res.results[c]["emitacc"].astype(np.float64)         # [128, 2*BL]
        for bl in range(BL):
            b = c * BL + bl
            logZ[b] = np.log(zh[slot_of[int(tstar[b])], bl]) + tstar[b] * SIGMA
            emit[b] = ea[:, bl].sum() + ea[:, BL + bl].sum()

    gold = emit + _host_gold_small(targets, lengths, transitions,
                                   head_transitions, last_transitions)
    return (logZ - gold).astype(np.float32)[:, None]              # [B, C=1]


def _host_exact(emissions, targets, lengths, transitions, head_transitions,
                last_transitions, idx):
    from scipy.special import logsumexp  # noqa — tiny fallback, rarely used
    outs = []
    T = transitions[0].astype(np.float64)
    for b in idx:
        L = int(lengths[b])
        em = emissions[b, :, 0, :].astype(np.float64)
        alpha = head_transitions[0].astype(np.float64) + em[0]
        for t in range(1, L):
            alpha = logsumexp(alpha[:, None] + T, axis=0) + em[t]
        logZ = logsumexp(alpha + last_transitions[0].astype(np.float64))
        tgt = targets[b]
        gold = em[np.arange(L), tgt[:L]].sum()
        gold += T[tgt[:L - 1], tgt[1:L]].sum()
        gold += head_transitions[0][tgt[0]] + last_transitions[0][tgt[L - 1]]
        outs.append(logZ - gold)
    return np.asarray(outs, np.float32)


# revision 25
# speedup vs baseline: 1.0968x; 1.0008x over previous
"""CRF decoder (logZ - gold) Trainium2 kernel.

Strategy (hardcoded for B=64, S=1024, C=1, N=256, 8 cores):
- Data-parallel over batch: 8 sequences per core.
- Log-semiring forward scan done in *linear* space with a constant host-side
  log-scale sigma = log(256)+0.5 subtracted from each emission, so the scaled
  probabilities p_t stay within fp32/bf16 exponent range for all 1024 steps
  (drift is a mean-zero random walk, ~±3 nats) — no device renormalization.
- Per step: u = W^T p (4 bf16 128x128 matmuls, PSUM fp32), p' = u * E_t (DVE),
  where W = exp(transitions), E_t = exp(em_t - sigma) (ScalarE bulk exp).
- Variable lengths: per-step scalar z_t[b] = p_t . exp(last) via a 1-column
  matmul; host reads z at t = len_b - 1 and assembles
  logZ_b = log z_{len-1} + (len-1)*sigma.  No per-step masking on device.
- Gold emission score on device: one-hot (host-built, masked) times raw
  emissions, multiply+reduce per chunk on DVE, partition-sum on host (tiny).
- Gold transition/head/last scores touch only the tiny parameter tensors and
  targets; computed on host.
"""

import math
from contextlib import ExitStack

import numpy as np
import ml_dtypes

import concourse.bass as bass
import concourse.tile as tile
from concourse import bacc, mybir
from concourse.bass_utils import run_bass_kernel_spmd

B, S, N = 64, 1024, 256
NCORES = 8
BL = B // NCORES  # 8 sequences per core
TC = 128          # time-chunk length
NCHUNK = S // TC
SIGMA = math.log(256.0) + 0.5
ZMIN = 383        # earliest t for which z_t is recorded (lengths >= ZMIN+2 expected)

F32 = mybir.dt.float32
BF16 = mybir.dt.bfloat16


def _crf_tile_kernel(ctx: ExitStack, tc: tile.TileContext, aps: dict,
                     tstars: tuple):
    nc = tc.nc
    em_d, oh_d = aps["em"], aps["oh"]          # [2,128,S,BL] bf16 dram
    w_d = aps["w"]                              # [2,128,2,128] bf16
    el_d = aps["el"]                            # [2,128,1] bf16
    hd_d = aps["hd"]                            # [2,128,1] f32
    zh_d = aps["zhist"]                         # [1, S*BL] f32 out
    ea_d = aps["emitacc"]                       # [128, 2*BL] f32 out

    consts = ctx.enter_context(tc.tile_pool(name="consts", bufs=1))
    state = ctx.enter_context(tc.tile_pool(name="state", bufs=1))
    empool = ctx.enter_context(tc.tile_pool(name="em", bufs=4))
    ohpool = ctx.enter_context(tc.tile_pool(name="oh", bufs=4))
    epool = ctx.enter_context(tc.tile_pool(name="E", bufs=4))
    tmppool = ctx.enter_context(tc.tile_pool(name="tmp", bufs=2))
    redpool = ctx.enter_context(tc.tile_pool(name="red", bufs=2))
    upool = ctx.enter_context(tc.tile_pool(name="u", bufs=3, space="PSUM"))
    zpool = ctx.enter_context(tc.tile_pool(name="z", bufs=2, space="PSUM"))

    # ---- constants into SBUF ----
    w_sb = []   # w_sb[ih][:, jh, :] = W[ih*128:(ih+1)*128, jh*128:(jh+1)*128]
    for ih in range(2):
        t_ = consts.tile([128, 2, 128], BF16, name=f"w{ih}", tag=f"w{ih}")
        nc.sync.dma_start(out=t_[:], in_=w_d[ih])
        w_sb.append(t_)
    el_sb = []
    hd_sb = []
    for ih in range(2):
        e_ = consts.tile([128, 1], BF16, name=f"el{ih}", tag=f"el{ih}")
        nc.sync.dma_start(out=e_[:], in_=el_d[ih])
        el_sb.append(e_)
        h_ = consts.tile([128, 1], F32, name=f"hd{ih}", tag=f"hd{ih}")
        nc.sync.dma_start(out=h_[:], in_=hd_d[ih])
        hd_sb.append(h_)

    sig_sb = consts.tile([128, 1], F32, name="sigb", tag="sigb")
    nc.vector.memset(sig_sb[:], -SIGMA)

    # persistent state: ping-pong p tiles [128, (jh, b)], per group
    GB = BL // 2
    p_sb = [[state.tile([128, 2, GB], BF16, name=f"p{par}g{g}", tag=f"p{par}g{g}")
             for g in range(2)] for par in range(2)]
    # z snapshots: one [1,BL] slot per distinct snapshot step (all cores
    # write every slot; host picks its core's column at its length's slot)
    nslots = max(len(tstars), 1)
    zhist = consts.tile([1, nslots * BL], F32, name="zhist", tag="zhist")
    acc = consts.tile([128, 2, BL], F32, name="acc", tag="acc")
    nc.vector.memset(acc[:], 0.0)

    alu = mybir.AluOpType

    def do_z(slot, p_pair):
        """z = expLast . p -> zhist[slot] (both groups)"""
        for g in range(2):
            z = zpool.tile([1, GB], F32, name="z", tag="z")
            nc.tensor.matmul(z[:], el_sb[0][:], p_pair[g][:, 0, :], start=True, stop=False)
            nc.tensor.matmul(z[:], el_sb[1][:], p_pair[g][:, 1, :], start=False, stop=True)
            nc.scalar.copy(
                zhist[:, slot * BL + g * GB: slot * BL + (g + 1) * GB], z[:])

    # tstars is the sorted union of distinct snapshot steps; slot = index
    zsteps = {int(t_): k for k, t_ in enumerate(tstars)}

    for c in range(NCHUNK):
        em_t = empool.tile([128, TC, 2, BL], BF16, name="emt", tag="em")
        nc.sync.dma_start(out=em_t[:], in_=em_d[:, c * TC:(c + 1) * TC, :, :])
        oh_t = ohpool.tile([128, TC, 2, BL], BF16, name="oht", tag="oh")
        nc.sync.dma_start(out=oh_t[:], in_=oh_d[:, c * TC:(c + 1) * TC, :, :])
        e_t = epool.tile([128, TC, 2, BL], BF16, name="Et", tag="E")
        nc.scalar.activation(e_t[:], em_t[:],
                             mybir.ActivationFunctionType.Exp,
                             bias=sig_sb[:], scale=1.0)

        if c == 0:
            # p_0 = exp(head + em_0)
            for g in range(2):
                for ih in range(2):
                    nc.scalar.activation(
                        p_sb[0][g][:, ih, :],
                        em_t[:, 0, ih, g * GB:(g + 1) * GB],
                        mybir.ActivationFunctionType.Exp,
                        bias=hd_sb[ih][:], scale=1.0)
            if 0 in zsteps:
                do_z(zsteps[0], p_sb[0])

        # ---- emission gold score for this chunk (DVE, off critical path) ----
        tmp = tmppool.tile([128, TC, 2, BL], BF16, name="tmpt", tag="tmp")
        nc.gpsimd.tensor_mul(tmp[:], em_t[:], oh_t[:])
        red = redpool.tile([128, 2, BL], F32, name="redt", tag="red")
        nc.vector.tensor_reduce(red[:], tmp[:].rearrange("p t h b -> p h b t"),
                                mybir.AxisListType.X, alu.add)
        nc.vector.tensor_add(acc[:], acc[:], red[:])

        # ---- the scan steps of this chunk ----
        for r in range(TC):
            t = c * TC + r
            if t == 0:
                continue
            pa = p_sb[(t - 1) % 2]
            pb = p_sb[t % 2]
            # two independent sequence groups: group B's matmuls fill the
            # PE while group A's vector multiply + sync latency elapse.
            # Weight-tile orders arranged so consecutive bursts start with
            # the tile the previous burst ended on (A fwd, B reversed).
            us = []
            for g in range(2):
                u = upool.tile([128, 2, GB], F32, name=f"u{g}", tag=f"u{g}")
                us.append(u)
                p_ = pa[g]
                if g == 0:
                    nc.tensor.matmul(u[:, 0, :], w_sb[0][:, 0, :], p_[:, 0, :], start=True, stop=False)
                    nc.tensor.matmul(u[:, 0, :], w_sb[1][:, 0, :], p_[:, 1, :], start=False, stop=True)
                    nc.tensor.matmul(u[:, 1, :], w_sb[0][:, 1, :], p_[:, 0, :], start=True, stop=False)
                    nc.tensor.matmul(u[:, 1, :], w_sb[1][:, 1, :], p_[:, 1, :], start=False, stop=True)
                else:
                    nc.tensor.matmul(u[:, 1, :], w_sb[1][:, 1, :], p_[:, 1, :], start=True, stop=False)
                    nc.tensor.matmul(u[:, 1, :], w_sb[0][:, 1, :], p_[:, 0, :], start=False, stop=True)
                    nc.tensor.matmul(u[:, 0, :], w_sb[1][:, 0, :], p_[:, 1, :], start=True, stop=False)
                    nc.tensor.matmul(u[:, 0, :], w_sb[0][:, 0, :], p_[:, 0, :], start=False, stop=True)
            for g in range(2):
                nc.vector.tensor_mul(pb[g][:], us[g][:],
                                     e_t[:, r, :, g * GB:(g + 1) * GB])
            if t in zsteps:
                do_z(zsteps[t], pb)

    # ---- outputs ----
    nc.sync.dma_start(out=zh_d[:], in_=zhist[:])
    nc.sync.dma_start(out=ea_d[:], in_=acc[:].rearrange("p h b -> p (h b)"))


_NC_CACHE = {}


def _build_nc(tstars=(S - 1,)):
    """tstars: sorted union (over all cores/sequences) of snapshot steps
    len_b - 1. SPMD — the single shared program snapshots z at every such
    step into its own slot; each core's host-side assembly picks its column.
    """
    key = tuple(tstars)
    if key in _NC_CACHE:
        return _NC_CACHE[key]
    nc = bacc.Bacc("TRN2", target_bir_lowering=False, debug=False,
                   num_devices=NCORES)
    aps = {
        "em": nc.dram_tensor("em", [128, S, 2, BL], BF16, kind="ExternalInput").ap(),
        "oh": nc.dram_tensor("oh", [128, S, 2, BL], BF16, kind="ExternalInput").ap(),
        "w": nc.dram_tensor("w", [2, 128, 2, 128], BF16, kind="ExternalInput").ap(),
        "el": nc.dram_tensor("el", [2, 128, 1], BF16, kind="ExternalInput").ap(),
        "hd": nc.dram_tensor("hd", [2, 128, 1], F32, kind="ExternalInput").ap(),
        "zhist": nc.dram_tensor("zhist", [1, max(len(tstars), 1) * BL], F32,
                                kind="ExternalOutput").ap(),
        "emitacc": nc.dram_tensor("emitacc", [128, 2 * BL], F32, kind="ExternalOutput").ap(),
    }
    with tile.TileContext(nc) as tc:
        with ExitStack() as ctx:
            _crf_tile_kernel(ctx, tc, aps, tuple(tstars))
    nc.compile()
    _NC_CACHE[key] = nc
    return nc


def _host_gold_small(targets, lengths, transitions, head_transitions, last_transitions):
    """Transition/head/last parts of the gold score (no big-tensor access)."""
    T = transitions[0].astype(np.float64)
    tr = T[targets[:, :-1], targets[:, 1:]]                       # [B,S-1]
    pmask = (np.arange(1, S)[None, :] < lengths[:, None])
    trans_score = (tr * pmask).sum(1)
    head_score = head_transitions[0][targets[:, 0]].astype(np.float64)
    last_tag = np.take_along_axis(targets, (lengths - 1)[:, None], axis=1)[:, 0]
    last_score = last_transitions[0][last_tag].astype(np.float64)
    return trans_score + head_score + last_score


def _make_in_maps(inputs):
    emissions = np.asarray(inputs["emissions"])
    targets = np.asarray(inputs["targets"])
    lengths = np.asarray(inputs["lengths"])
    transitions = np.asarray(inputs["transitions"])
    head_transitions = np.asarray(inputs["head_transitions"])
    last_transitions = np.asarray(inputs["last_transitions"])

    W = np.exp(transitions[0].astype(np.float64)).astype(ml_dtypes.bfloat16)
    w_sh = np.ascontiguousarray(W.reshape(2, 128, 2, 128))
    el_sh = np.ascontiguousarray(
        np.exp(last_transitions[0].astype(np.float64))
        .astype(ml_dtypes.bfloat16).reshape(2, 128, 1))
    hd_sh = np.ascontiguousarray(
        head_transitions[0].astype(np.float32).reshape(2, 128, 1))

    em_bf = emissions[:, :, 0, :].astype(ml_dtypes.bfloat16)      # [B,S,N]

    in_maps = []
    for c in range(NCORES):
        sl = slice(c * BL, (c + 1) * BL)
        em_c = np.ascontiguousarray(
            em_bf[sl].transpose(2, 1, 0).reshape(2, 128, S, BL)
            .transpose(1, 2, 0, 3))                   # [jlo, t, jh, b]
        tgt_c = targets[sl]                                       # [BL,S]
        len_c = lengths[sl]
        oh_c = np.zeros((N, S, BL), dtype=ml_dtypes.bfloat16)
        bb, tt = np.meshgrid(np.arange(BL), np.arange(S), indexing="ij")
        valid = tt < len_c[:, None]
        oh_c[tgt_c[bb[valid], tt[valid]], tt[valid], bb[valid]] = 1.0
        oh_c = np.ascontiguousarray(
            oh_c.reshape(2, 128, S, BL).transpose(1, 2, 0, 3))
        in_maps.append({"em": em_c, "oh": oh_c, "w": w_sh, "el": el_sh,
                        "hd": hd_sh})
    return in_maps


def kernel(emissions, targets, lengths, transitions, head_transitions,
           last_transitions):
    emissions = np.asarray(emissions)
    targets = np.asarray(targets)
    lengths = np.asarray(lengths)
    transitions = np.asarray(transitions)
    head_transitions = np.asarray(head_transitions)
    last_transitions = np.asarray(last_transitions)
    assert emissions.shape == (B, S, 1, N), emissions.shape

    tstar = np.clip(lengths - 1, 0, S - 1)
    tstars = tuple(sorted(set(int(t) for t in tstar)))
    nc = _build_nc(tstars)
    slot_of = {t: k for k, t in enumerate(tstars)}
    in_maps = _make_in_maps(dict(
        emissions=emissions, targets=targets, lengths=lengths,
        transitions=transitions, head_transitions=head_transitions,
        last_transitions=last_transitions))

    res = run_bass_kernel_spmd(nc, in_maps, list(range(NCORES)))

    logZ = np.zeros(B, np.float64)
    emit = np.zeros(B, np.float64)
    for c in range(NCORES):
        zh = res.results[c]["zhist"].reshape(len(tstars), BL).astype(np.float64)
        ea = res.results[c]["emitacc"].astype(np.float64)         # [128, 2*BL]
        for bl in range(BL):
            b = c * BL + bl
            logZ[b] = np.log(zh[slot_of[int(tstar[b])], bl]) + tstar[b] * SIGMA
            emit[b] = ea[:, bl].sum() + ea[:, BL + bl].sum()

    gold = emit + _host_gold_small(targets, lengths, transitions,
                                   head_transitions, last_transitions)
    return (logZ - gold).astype(np.float32)[:, None]              # [B, C=1]


def _host_exact(emissions, targets, lengths, transitions, head_transitions,
                last_transitions, idx):
    from scipy.special import logsumexp  # noqa — tiny fallback, rarely used
    outs = []
    T = transitions[0].astype(np.float64)
    for b in idx:
        L = int(lengths[b])
        em = emissions[b, :, 0, :].astype(np.float64)
        alpha = head_transitions[0].astype(np.float64) + em[0]
        for t in range(1, L):
            alpha = logsumexp(alpha[:, None] + T, axis=0) + em[t]
        logZ = logsumexp(alpha + last_transitions[0].astype(np.float64))
        tgt = targets[b]
        gold = em[np.arange(L), tgt[:L]].sum()
        gold += T[tgt[:L - 1], tgt[1:L]].sum()
        gold += head_transitions[0][tgt[0]] + last_transitions[0][tgt[L - 1]]
        outs.append(logZ - gold)
    return np.asarray(outs, np.float32)
